# revision 3
# baseline (speedup 1.0000x reference)
"""Trainium2 Bass kernel for the 32-block Feistel CNN (nn_Core_70325794505291).

Strategy: data-parallel over batch (8 elements -> 8 cores). Each core runs the
full 192-conv + 32-mix tower on one batch element.

conv3x3 = 9 shifted matmuls (taps) accumulating into fp32 PSUM. Matmul operands
are fp16 (1 cycle/row on the PE); persistent activations stay fp32 in SBUF.
Activations are rescaled by 1/G at each block boundary (the tower amplifies
~1.7x/block which would overflow fp16); biases are pre-scaled on the host and
conv3 biases are folded into the preceding block-boundary epilogue, so the
rescale is free.  End-to-end rel-err vs the fp32 reference ~2.4e-3 (simulated).

Host-side work is layout only (permutation gather, space-to-depth, weight
repacking); every FLOP of the model runs on-device.
"""

import os

import numpy as np

# ---- static problem structure (mirrors the reference tape) ----
BLOCKS = 32
BLOCK_SIZE = 8
CONV_SHAPES = [(128, 96), (128, 128), (96, 128)] * 2
H = W = 32
HW = H * W
N_CHUNK = 512  # matmul free-dim chunk (one PSUM bank)

# fp16 scale schedule
G = 1.7
S0 = 32.0

# per-block weight blob column layout (fp16 elements per partition row)
# round r at base 3168*r: conv1 [96p, 1152] | conv2 [128p, 1152] | conv3 [128p, 864]
# mix at 6336: k1-oA [96p,96] | k1-oB | k2-oA | k2-oB
WCOLS = 6720
MIXBASE = 6336


def _conv_meta():
    meta = []
    w_total = 0
    b_total = 0
    for _ in range(BLOCKS):
        for oc, ic in CONV_SHAPES:
            meta.append((oc, ic, w_total, b_total))
            w_total += oc * ic * 9
            b_total += oc
    return meta, w_total, b_total


CONV_META, W_TOTAL, B_TOTAL = _conv_meta()


def _nblocks():
    return int(os.environ.get("BASS_NN_NBLOCKS", BLOCKS))


# --------------------------------------------------------------------------
# host packing
# --------------------------------------------------------------------------

def pack_inputs(x, mu, sigma, w_flat, b_flat, m):
    """Build per-core and shared device input arrays."""
    nb = _nblocks()
    x = np.asarray(x, np.float32)
    mu = np.asarray(mu, np.float32)
    sigma = np.asarray(sigma, np.float32)
    w_flat = np.asarray(w_flat, np.float32)
    b_flat = np.asarray(b_flat, np.float32)
    m = np.asarray(m, np.float32)

    # weight blobs [nb, 128, WCOLS] fp16
    wblk = np.zeros((nb, 128, WCOLS), np.float16)
    for blk in range(nb):
        for r in range(2):
            base = 3168 * r
            ci = blk * 6 + r * 3
            for k, (colw, coln) in enumerate(((0, 1152), (1152, 1152), (2304, 864))):
                oc, ic, woff, _ = CONV_META[ci + k]
                arr = w_flat[woff:woff + oc * ic * 9].reshape(oc, ic, 9)
                panel = arr.transpose(1, 2, 0).reshape(ic, 9 * oc)
                wblk[blk, :ic, base + colw:base + colw + coln] = panel.astype(np.float16)
        mt = m[blk].T  # [c, o]
        wblk[blk, 0:96, MIXBASE + 0:MIXBASE + 96] = mt[0:96, 0:96].astype(np.float16)
        wblk[blk, 0:96, MIXBASE + 96:MIXBASE + 192] = mt[0:96, 96:192].astype(np.float16)
        wblk[blk, 0:96, MIXBASE + 192:MIXBASE + 288] = mt[96:192, 0:96].astype(np.float16)
        wblk[blk, 0:96, MIXBASE + 288:MIXBASE + 384] = mt[96:192, 96:192].astype(np.float16)

    def bias_of(ci):
        oc, _, _, boff = CONV_META[ci]
        return b_flat[boff:boff + oc]

    # bias blobs [nb, 128, 6] fp32
    bblk = np.zeros((nb, 128, 6), np.float32)
    for blk in range(nb):
        sk = S0 * G ** (-blk)
        bblk[blk, 0:128, 0] = bias_of(blk * 6 + 0) * sk
        bblk[blk, 0:128, 1] = bias_of(blk * 6 + 1) * sk
        bblk[blk, 0:128, 2] = bias_of(blk * 6 + 3) * sk
        bblk[blk, 0:128, 3] = bias_of(blk * 6 + 4) * sk
        if blk + 1 < nb:
            sk1 = S0 * G ** (-(blk + 1))
            bblk[blk, 0:96, 4] = bias_of((blk + 1) * 6 + 2) * sk1
            bblk[blk, 0:96, 5] = bias_of((blk + 1) * 6 + 5) * sk1

    # affine vectors [96, 4]: scaleA, biasA, scaleB, biasB
    affv = np.zeros((96, 4), np.float32)
    p = np.arange(96)
    cA = p // 64            # channels for partitions 0..95
    cB = (96 + p) // 64     # channels for partitions 96..191
    affv[:, 0] = S0 / sigma[cA]
    affv[:, 1] = -S0 * mu[cA] / sigma[cA] + S0 * bias_of(2)
    affv[:, 2] = S0 / sigma[cB]
    affv[:, 3] = -S0 * mu[cB] / sigma[cB] + S0 * bias_of(5)

    return wblk, bblk, affv


def pack_x(x, perm):
    """perm + space-to-depth layout reorg -> [B, 2, 96, 1024] fp32."""
    x = np.asarray(x, np.float32)
    perm = np.asarray(perm)
    B = x.shape[0]
    v = x.reshape(B, 3, 256 * 256)
    v = np.take_along_axis(v, perm[None].astype(np.int64), axis=2)
    v = v.reshape(B, 3, 32, 8, 32, 8)
    v = v.transpose(0, 1, 3, 5, 2, 4).reshape(B, 192, HW)
    return np.ascontiguousarray(v.reshape(B, 2, 96, HW))


# --------------------------------------------------------------------------
# bass program
# --------------------------------------------------------------------------

def build_bass(nb):
    import concourse.bacc as bacc
    import concourse.mybir as mybir
    import concourse.tile as tile
    from concourse.mybir import dt, ActivationFunctionType as AF, AluOpType as ALU

    nc = bacc.Bacc("TRN2", target_bir_lowering=False, debug=False,
                   enable_asserts=False, num_devices=8)

    xin = nc.dram_tensor("xin", [2, 96, HW], dt.float32, kind="ExternalInput").ap()
    affv_d = nc.dram_tensor("affv", [96, 4], dt.float32, kind="ExternalInput").ap()
    wblk_d = nc.dram_tensor("wblk", [nb, 128, WCOLS], dt.float16, kind="ExternalInput").ap()
    bblk_d = nc.dram_tensor("bblk", [nb, 128, 6], dt.float32, kind="ExternalInput").ap()
    yout = nc.dram_tensor("yout", [2, 96, HW], dt.float32, kind="ExternalOutput").ap()

    PADF = 34 * 34  # padded fp16 plane

    with tile.TileContext(nc) as tc:
        with (
            tc.tile_pool(name="wpool", bufs=3) as wpool,
            tc.tile_pool(name="bpool", bufs=3) as bpool,
            tc.tile_pool(name="fpool", bufs=1) as fpool,
            tc.tile_pool(name="spool", bufs=4) as spool,
            tc.tile_pool(name="iopool", bufs=1) as iopool,
            tc.tile_pool(name="pspool", bufs=3, space="PSUM") as pspool,
        ):
            # persistent zero-padded fp16 planes: [pbB, t1, t2, h1, h2]
            pads = []
            for i in range(5):
                pb = fpool.tile([128, PADF], dt.float16, name=f"pad{i}", tag=f"pad{i}")
                nc.vector.memset(pb, 0.0)
                pads.append(pb)

            def pv(i, p0=128):
                return pads[i].rearrange("c (h w) -> c h w", h=34)[0:p0]

            afft = iopool.tile([96, 4], dt.float32, name="afft")
            nc.sync.dma_start(out=afft, in_=affv_d)
            xA = spool.tile([96, HW], dt.float32, name="xA", tag="state")
            nc.sync.dma_start(out=xA, in_=xin[0])
            xB = spool.tile([96, HW], dt.float32, name="xB", tag="state")
            nc.sync.dma_start(out=xB, in_=xin[1])

            A32 = spool.tile([96, HW], dt.float32, name="A32", tag="state")
            nc.vector.tensor_scalar(A32, xA, afft[:, 0:1], afft[:, 1:2], ALU.mult, ALU.add)
            B32 = spool.tile([96, HW], dt.float32, name="B32", tag="state")
            nc.vector.tensor_scalar(B32, xB, afft[:, 2:3], afft[:, 3:4], ALU.mult, ALU.add)
            # fp16 copy of B into pad0 interior
            nc.scalar.activation(pv(0, 96)[:, 1:33, 1:33], B32.rearrange("c (h w) -> c h w", h=32), AF.Copy)

            def conv_mms(psum, wt, wcol, oc, ic, rhs_pad_idx):
                """9-tap conv matmuls into psum ([oc, HW] fp32 tile)."""
                psv = psum.rearrange("c (h w) -> c h w", h=32)
                src = pv(rhs_pad_idx, ic)
                for n in range(2):
                    y0 = 16 * n
                    for t in range(9):
                        dy, dx = divmod(t, 3)
                        nc.tensor.matmul(
                            psv[:, y0:y0 + 16, :],
                            wt[0:ic, wcol + t * oc: wcol + (t + 1) * oc],
                            src[:, y0 + dy:y0 + dy + 16, dx:dx + 32],
                            start=(t == 0), stop=(t == 8),
                        )

            def relu_to_pad(psum, oc, pad_idx, bias_ap):
                """relu(psum + bias) -> fp16 interior of pads[pad_idx], split rows."""
                psv = psum.rearrange("c (h w) -> c h w", h=32)
                dst = pv(pad_idx, oc)
                for r0, r1 in ((0, 17), (17, 32)):
                    nc.scalar.activation(
                        dst[:, 1 + r0:1 + r1, 1:33], psv[:, r0:r1, :],
                        AF.Relu, bias=bias_ap,
                    )

            PB, T1, T2, H1, H2 = 0, 1, 2, 3, 4

            for blk in range(nb):
                wt = wpool.tile([128, WCOLS], dt.float16, name="wt", tag="wt")
                nc.sync.dma_start(out=wt, in_=wblk_d[blk])
                bt = bpool.tile([128, 6], dt.float32, name="bt", tag="bt")
                nc.sync.dma_start(out=bt, in_=bblk_d[blk])

                src_idx = PB
                for r in range(2):
                    base = 3168 * r
                    ps1 = pspool.tile([128, HW], dt.float32, name="ps1", tag="ps")
                    conv_mms(ps1, wt, base, 128, 96, src_idx)
                    relu_to_pad(ps1, 128, T1, bt[0:128, 2 * r:2 * r + 1])

                    ps2 = pspool.tile([128, HW], dt.float32, name="ps2", tag="ps")
                    conv_mms(ps2, wt, base + 1152, 128, 128, T1)
                    relu_to_pad(ps2, 128, T2, bt[0:128, 2 * r + 1:2 * r + 2])

                    ps3 = pspool.tile([96, HW], dt.float32, name="ps3", tag="ps")
                    conv_mms(ps3, wt, base + 2304, 96, 128, T2)

                    An = spool.tile([96, HW], dt.float32, name="An", tag="state")
                    nc.vector.tensor_add(An, ps3, A32)
                    hidx = H1 if r == 0 else H2
                    nc.scalar.activation(
                        pv(hidx, 96)[:, 1:33, 1:33],
                        An.rearrange("c (h w) -> c h w", h=32), AF.Copy)
                    A32, B32 = B32, An
                    src_idx = hidx

                # mix: out halves A (o 0:96) and B (o 96:192)
                psA = pspool.tile([96, HW], dt.float32, name="psA", tag="ps")
                psB = pspool.tile([96, HW], dt.float32, name="psB", tag="ps")
                for ps, ocol in ((psA, 0), (psB, 96)):
                    psv = ps.rearrange("c (h w) -> c h w", h=32)
                    for n in range(2):
                        y0 = 16 * n
                        nc.tensor.matmul(
                            psv[:, y0:y0 + 16, :],
                            wt[0:96, MIXBASE + ocol:MIXBASE + ocol + 96],
                            pv(H1, 96)[:, 1 + y0:17 + y0, 1:33],
                            start=True, stop=False)
                        nc.tensor.matmul(
                            psv[:, y0:y0 + 16, :],
                            wt[0:96, MIXBASE + 192 + ocol:MIXBASE + 192 + ocol + 96],
                            pv(H2, 96)[:, 1 + y0:17 + y0, 1:33],
                            start=False, stop=True)

                if blk + 1 < nb:
                    gamma = 1.0 / G
                    A32 = spool.tile([96, HW], dt.float32, name="mA", tag="state")
                    nc.vector.tensor_scalar(A32, psA, gamma, bt[0:96, 4:5], ALU.mult, ALU.add)
                    B32 = spool.tile([96, HW], dt.float32, name="mB", tag="state")
                    nc.vector.tensor_scalar(B32, psB, gamma, bt[0:96, 5:6], ALU.mult, ALU.add)
                    nc.scalar.activation(pv(PB, 96)[:, 1:33, 1:33],
                                         B32.rearrange("c (h w) -> c h w", h=32), AF.Copy)
                else:
                    gamma = float(G ** (nb - 1) / S0)
                    oA = spool.tile([96, HW], dt.float32, name="oA", tag="state")
                    nc.vector.tensor_scalar(oA, psA, gamma, None, ALU.mult)
                    oB = spool.tile([96, HW], dt.float32, name="oB", tag="state")
                    nc.vector.tensor_scalar(oB, psB, gamma, None, ALU.mult)
                    nc.sync.dma_start(out=yout[0], in_=oA)
                    nc.sync.dma_start(out=yout[1], in_=oB)

    nc.compile()
    return nc


# --------------------------------------------------------------------------
# entry point
# --------------------------------------------------------------------------

_last_results = None


def kernel(x, mu, sigma, w_flat, b_flat, m, perm, ops):
    global _last_results
    from concourse.bass_utils import run_bass_kernel_spmd

    nb = _nblocks()
    x = np.asarray(x)
    B = x.shape[0]
    n_cores = 8
    assert B == n_cores, f"expected batch 8, got {B}"

    wblk, bblk, affv = pack_inputs(x, mu, sigma, w_flat, b_flat, m)
    xs = pack_x(x, perm)

    nc = build_bass(nb)

    in_maps = []
    for b in range(B):
        in_maps.append({
            "xin": np.ascontiguousarray(xs[b]),
            "affv": affv,
            "wblk": wblk,
            "bblk": bblk,
        })

    trace = bool(int(os.environ.get("BASS_NN_TRACE", "0")))
    res = run_bass_kernel_spmd(nc, in_maps, core_ids=list(range(n_cores)),
                               trace=trace)
    _last_results = res

    out = np.empty((B, 192, 32, 32), np.float32)
    for b in range(B):
        y = res.results[b]["yout"]  # [2, 96, HW]
        out[b] = y.reshape(192, 32, 32)
    return out


# revision 4
# speedup vs baseline: 1.6404x; 1.6404x over previous
"""Trainium2 Bass kernel for the 32-block Feistel CNN (nn_Core_70325794505291).

Strategy: data-parallel over batch (8 elements -> 8 cores). Each core runs the
full 192-conv + 32-mix tower on one batch element.

conv3x3 = 9 shifted matmuls (taps) accumulating into fp32 PSUM. Matmul operands
are fp16 (1 cycle/row on the PE); persistent activations stay fp32 in SBUF.
Activations are rescaled by 1/G at each block boundary (the tower amplifies
~1.7x/block which would overflow fp16); biases are pre-scaled on the host and
conv3 biases are folded into the preceding block-boundary epilogue, so the
rescale is free.  End-to-end rel-err vs the fp32 reference ~2.4e-3 (simulated).

Host-side work is layout only (permutation gather, space-to-depth, weight
repacking); every FLOP of the model runs on-device.
"""

import os

import numpy as np

# ---- static problem structure (mirrors the reference tape) ----
BLOCKS = 32
BLOCK_SIZE = 8
CONV_SHAPES = [(128, 96), (128, 128), (96, 128)] * 2
H = W = 32
HW = H * W
N_CHUNK = 512  # matmul free-dim chunk (one PSUM bank)

# fp16 scale schedule
G = 1.7
S0 = 32.0

# per-block weight blob column layout (fp16 elements per partition row)
# round r at base 3168*r: conv1 [96p, 1152] | conv2 [128p, 1152] | conv3 [128p, 864]
# mix at 6336: k1-oA [96p,96] | k1-oB | k2-oA | k2-oB
WCOLS = 6720
MIXBASE = 6336


def _conv_meta():
    meta = []
    w_total = 0
    b_total = 0
    for _ in range(BLOCKS):
        for oc, ic in CONV_SHAPES:
            meta.append((oc, ic, w_total, b_total))
            w_total += oc * ic * 9
            b_total += oc
    return meta, w_total, b_total


CONV_META, W_TOTAL, B_TOTAL = _conv_meta()


def _nblocks():
    return int(os.environ.get("BASS_NN_NBLOCKS", BLOCKS))


# --------------------------------------------------------------------------
# host packing
# --------------------------------------------------------------------------

def pack_inputs(x, mu, sigma, w_flat, b_flat, m):
    """Build per-core and shared device input arrays."""
    nb = _nblocks()
    x = np.asarray(x, np.float32)
    mu = np.asarray(mu, np.float32)
    sigma = np.asarray(sigma, np.float32)
    w_flat = np.asarray(w_flat, np.float32)
    b_flat = np.asarray(b_flat, np.float32)
    m = np.asarray(m, np.float32)

    # weight blobs [nb, 128, WCOLS] fp16
    wblk = np.zeros((nb, 128, WCOLS), np.float16)
    for blk in range(nb):
        for r in range(2):
            base = 3168 * r
            ci = blk * 6 + r * 3
            for k, (colw, coln) in enumerate(((0, 1152), (1152, 1152), (2304, 864))):
                oc, ic, woff, _ = CONV_META[ci + k]
                arr = w_flat[woff:woff + oc * ic * 9].reshape(oc, ic, 9)
                panel = arr.transpose(1, 2, 0).reshape(ic, 9 * oc)
                wblk[blk, :ic, base + colw:base + colw + coln] = panel.astype(np.float16)
        mt = m[blk].T  # [c, o]
        wblk[blk, 0:96, MIXBASE + 0:MIXBASE + 96] = mt[0:96, 0:96].astype(np.float16)
        wblk[blk, 0:96, MIXBASE + 96:MIXBASE + 192] = mt[0:96, 96:192].astype(np.float16)
        wblk[blk, 0:96, MIXBASE + 192:MIXBASE + 288] = mt[96:192, 0:96].astype(np.float16)
        wblk[blk, 0:96, MIXBASE + 288:MIXBASE + 384] = mt[96:192, 96:192].astype(np.float16)

    def bias_of(ci):
        oc, _, _, boff = CONV_META[ci]
        return b_flat[boff:boff + oc]

    # bias blobs [nb, 128, 6] fp32
    bblk = np.zeros((nb, 128, 6), np.float32)
    for blk in range(nb):
        sk = S0 * G ** (-blk)
        bblk[blk, 0:128, 0] = bias_of(blk * 6 + 0) * sk
        bblk[blk, 0:128, 1] = bias_of(blk * 6 + 1) * sk
        bblk[blk, 0:128, 2] = bias_of(blk * 6 + 3) * sk
        bblk[blk, 0:128, 3] = bias_of(blk * 6 + 4) * sk
        if blk + 1 < nb:
            sk1 = S0 * G ** (-(blk + 1))
            bblk[blk, 0:96, 4] = bias_of((blk + 1) * 6 + 2) * sk1
            bblk[blk, 0:96, 5] = bias_of((blk + 1) * 6 + 5) * sk1

    # affine vectors [96, 4]: scaleA, biasA, scaleB, biasB
    affv = np.zeros((96, 4), np.float32)
    p = np.arange(96)
    cA = p // 64            # channels for partitions 0..95
    cB = (96 + p) // 64     # channels for partitions 96..191
    affv[:, 0] = S0 / sigma[cA]
    affv[:, 1] = -S0 * mu[cA] / sigma[cA] + S0 * bias_of(2)
    affv[:, 2] = S0 / sigma[cB]
    affv[:, 3] = -S0 * mu[cB] / sigma[cB] + S0 * bias_of(5)

    return wblk, bblk, affv


def pack_x(x, perm):
    """perm + space-to-depth layout reorg -> [B, 2, 96, 1024] fp32."""
    x = np.asarray(x, np.float32)
    perm = np.asarray(perm)
    B = x.shape[0]
    v = x.reshape(B, 3, 256 * 256)
    v = np.take_along_axis(v, perm[None].astype(np.int64), axis=2)
    v = v.reshape(B, 3, 32, 8, 32, 8)
    v = v.transpose(0, 1, 3, 5, 2, 4).reshape(B, 192, HW)
    return np.ascontiguousarray(v.reshape(B, 2, 96, HW))


# --------------------------------------------------------------------------
# bass program
# --------------------------------------------------------------------------

def build_bass(nb):
    import concourse.bacc as bacc
    import concourse.mybir as mybir
    import concourse.tile as tile
    from concourse.mybir import dt, ActivationFunctionType as AF, AluOpType as ALU

    nc = bacc.Bacc("TRN2", target_bir_lowering=False, debug=False,
                   enable_asserts=False, num_devices=8)

    xin = nc.dram_tensor("xin", [2, 96, HW], dt.float32, kind="ExternalInput").ap()
    affv_d = nc.dram_tensor("affv", [96, 4], dt.float32, kind="ExternalInput").ap()
    wblk_d = nc.dram_tensor("wblk", [nb, 128, WCOLS], dt.float16, kind="ExternalInput").ap()
    bblk_d = nc.dram_tensor("bblk", [nb, 128, 6], dt.float32, kind="ExternalInput").ap()
    yout = nc.dram_tensor("yout", [2, 96, HW], dt.float32, kind="ExternalOutput").ap()

    PADF = 34 * 34  # padded fp16 plane

    with tile.TileContext(nc) as tc:
        with (
            tc.tile_pool(name="wpool", bufs=3) as wpool,
            tc.tile_pool(name="bpool", bufs=3) as bpool,
            tc.tile_pool(name="fpool", bufs=1) as fpool,
            tc.tile_pool(name="spool", bufs=4) as spool,
            tc.tile_pool(name="iopool", bufs=1) as iopool,
            tc.tile_pool(name="pspool", bufs=6, space="PSUM") as pspool,
        ):
            # persistent zero-padded fp16 planes: [pbB, t1, t2, h1, h2]
            pads = []
            for i in range(5):
                pb = fpool.tile([128, PADF], dt.float16, name=f"pad{i}", tag=f"pad{i}")
                nc.vector.memset(pb, 0.0)
                pads.append(pb)

            def pv(i, p0=128):
                return pads[i].rearrange("c (h w) -> c h w", h=34)[0:p0]

            afft = iopool.tile([96, 4], dt.float32, name="afft")
            nc.sync.dma_start(out=afft, in_=affv_d)
            xA = spool.tile([96, HW], dt.float32, name="xA", tag="state")
            nc.sync.dma_start(out=xA, in_=xin[0])
            xB = spool.tile([96, HW], dt.float32, name="xB", tag="state")
            nc.sync.dma_start(out=xB, in_=xin[1])

            A32 = spool.tile([96, HW], dt.float32, name="A32", tag="state")
            nc.vector.tensor_scalar(A32, xA, afft[:, 0:1], afft[:, 1:2], ALU.mult, ALU.add)
            B32 = spool.tile([96, HW], dt.float32, name="B32", tag="state")
            nc.vector.tensor_scalar(B32, xB, afft[:, 2:3], afft[:, 3:4], ALU.mult, ALU.add)
            # fp16 copy of B into pad0 interior
            nc.scalar.activation(pv(0, 96)[:, 1:33, 1:33], B32.rearrange("c (h w) -> c h w", h=32), AF.Copy)

            def conv_mms(ps_pair, wt, wcol, oc, ic, rhs_pad_idx):
                """9-tap conv matmuls; chunk n goes to its own 1-bank psum tile."""
                src = pv(rhs_pad_idx, ic)
                for n, ps in enumerate(ps_pair):
                    psv = ps.rearrange("c (h w) -> c h w", h=16)
                    y0 = 16 * n
                    for t in range(9):
                        dy, dx = divmod(t, 3)
                        nc.tensor.matmul(
                            psv[:, :, :],
                            wt[0:ic, wcol + t * oc: wcol + (t + 1) * oc],
                            src[:, y0 + dy:y0 + dy + 16, dx:dx + 32],
                            start=(t == 0), stop=(t == 8),
                        )

            def relu_to_pad(ps_pair, oc, pad_idx, bias_ap):
                """relu(psum + bias) -> fp16 pad interior. 3-way split: ACT takes
                rows 0-15 (early, overlaps chunk1 MMs) + the row-16 sliver so the
                next conv's chunk0 unblocks fast; DVE takes rows 17-31 in
                parallel."""
                va = ps_pair[0].rearrange("c (h w) -> c h w", h=16)
                vb = ps_pair[1].rearrange("c (h w) -> c h w", h=16)
                dst = pv(pad_idx, oc)
                nc.scalar.activation(dst[:, 1:17, 1:33], va, AF.Relu, bias=bias_ap)
                nc.scalar.activation(dst[:, 17:18, 1:33], vb[:, 0:1, :], AF.Relu, bias=bias_ap)
                nc.vector.tensor_scalar(dst[:, 18:33, 1:33], vb[:, 1:16, :],
                                        bias_ap, 0.0, ALU.add, ALU.max)

            PB, T1, T2, H1, H2 = 0, 1, 2, 3, 4

            for blk in range(nb):
                wt = wpool.tile([128, WCOLS], dt.float16, name="wt", tag="wt")
                nc.sync.dma_start(out=wt, in_=wblk_d[blk])
                bt = bpool.tile([128, 6], dt.float32, name="bt", tag="bt")
                nc.sync.dma_start(out=bt, in_=bblk_d[blk])

                src_idx = PB
                for r in range(2):
                    base = 3168 * r
                    ps1 = [pspool.tile([128, N_CHUNK], dt.float32, name=f"ps1{n}", tag="ps")
                           for n in range(2)]
                    conv_mms(ps1, wt, base, 128, 96, src_idx)
                    relu_to_pad(ps1, 128, T1, bt[0:128, 2 * r:2 * r + 1])

                    ps2 = [pspool.tile([128, N_CHUNK], dt.float32, name=f"ps2{n}", tag="ps")
                           for n in range(2)]
                    conv_mms(ps2, wt, base + 1152, 128, 128, T1)
                    relu_to_pad(ps2, 128, T2, bt[0:128, 2 * r + 1:2 * r + 2])

                    ps3 = [pspool.tile([96, N_CHUNK], dt.float32, name=f"ps3{n}", tag="ps")
                           for n in range(2)]
                    conv_mms(ps3, wt, base + 2304, 96, 128, T2)

                    # Feistel add fused straight into the fp16 padded write
                    # (the f32 copy of the sum is never read again).
                    hidx = H1 if r == 0 else H2
                    dst = pv(hidx, 96)
                    v3a = ps3[0].rearrange("c (h w) -> c h w", h=16)
                    v3b = ps3[1].rearrange("c (h w) -> c h w", h=16)
                    A32v = A32.rearrange("c (h w) -> c h w", h=32)
                    nc.vector.tensor_add(dst[:, 1:17, 1:33], v3a, A32v[:, 0:16, :])
                    nc.vector.tensor_add(dst[:, 17:18, 1:33], v3b[:, 0:1, :], A32v[:, 16:17, :])
                    nc.vector.tensor_add(dst[:, 18:33, 1:33], v3b[:, 1:16, :], A32v[:, 17:32, :])
                    A32, B32 = B32, None
                    src_idx = hidx

                # mix: B half first (its fp16 copy gates the next block's conv1)
                def mix_mms(ocol):
                    pair = [pspool.tile([96, N_CHUNK], dt.float32, name=f"psm{n}", tag="ps")
                            for n in range(2)]
                    for n, ps in enumerate(pair):
                        psv = ps.rearrange("c (h w) -> c h w", h=16)
                        y0 = 16 * n
                        nc.tensor.matmul(
                            psv, wt[0:96, MIXBASE + ocol:MIXBASE + ocol + 96],
                            pv(H1, 96)[:, 1 + y0:17 + y0, 1:33], start=True, stop=False)
                        nc.tensor.matmul(
                            psv, wt[0:96, MIXBASE + 192 + ocol:MIXBASE + 192 + ocol + 96],
                            pv(H2, 96)[:, 1 + y0:17 + y0, 1:33], start=False, stop=True)
                    return pair

                psB = mix_mms(96)
                if blk + 1 < nb:
                    gamma = 1.0 / G
                    # fp16 copy for next conv1 straight off PSUM on ACT
                    dstB = pv(PB, 96)
                    for n, ps in enumerate(psB):
                        psv = ps.rearrange("c (h w) -> c h w", h=16)
                        nc.scalar.activation(dstB[:, 1 + 16 * n:17 + 16 * n, 1:33], psv,
                                             AF.Identity, bias=bt[0:96, 5:6], scale=gamma)
                    B32 = spool.tile([96, HW], dt.float32, name="mB", tag="state")
                    Bv = B32.rearrange("c (h w) -> c h w", h=32)
                    for n, ps in enumerate(psB):
                        nc.vector.tensor_scalar(Bv[:, 16 * n:16 * n + 16, :],
                                                ps.rearrange("c (h w) -> c h w", h=16),
                                                gamma, bt[0:96, 5:6], ALU.mult, ALU.add)
                    psA = mix_mms(0)
                    A32 = spool.tile([96, HW], dt.float32, name="mA", tag="state")
                    Av = A32.rearrange("c (h w) -> c h w", h=32)
                    for n, ps in enumerate(psA):
                        nc.vector.tensor_scalar(Av[:, 16 * n:16 * n + 16, :],
                                                ps.rearrange("c (h w) -> c h w", h=16),
                                                gamma, bt[0:96, 4:5], ALU.mult, ALU.add)
                else:
                    gamma = float(G ** (nb - 1) / S0)
                    psA = mix_mms(0)
                    oA = spool.tile([96, HW], dt.float32, name="oA", tag="state")
                    oB = spool.tile([96, HW], dt.float32, name="oB", tag="state")
                    for pair, ot in ((psA, oA), (psB, oB)):
                        ov = ot.rearrange("c (h w) -> c h w", h=32)
                        for n, ps in enumerate(pair):
                            nc.vector.tensor_scalar(ov[:, 16 * n:16 * n + 16, :],
                                                    ps.rearrange("c (h w) -> c h w", h=16),
                                                    gamma, None, ALU.mult)
                    nc.sync.dma_start(out=yout[0], in_=oA)
                    nc.sync.dma_start(out=yout[1], in_=oB)

    nc.compile()
    return nc


# --------------------------------------------------------------------------
# entry point
# --------------------------------------------------------------------------

_last_results = None


def kernel(x, mu, sigma, w_flat, b_flat, m, perm, ops):
    global _last_results
    from concourse.bass_utils import run_bass_kernel_spmd

    nb = _nblocks()
    x = np.asarray(x)
    B = x.shape[0]
    n_cores = 8
    assert B == n_cores, f"expected batch 8, got {B}"

    wblk, bblk, affv = pack_inputs(x, mu, sigma, w_flat, b_flat, m)
    xs = pack_x(x, perm)

    nc = build_bass(nb)

    in_maps = []
    for b in range(B):
        in_maps.append({
            "xin": np.ascontiguousarray(xs[b]),
            "affv": affv,
            "wblk": wblk,
            "bblk": bblk,
        })

    trace = bool(int(os.environ.get("BASS_NN_TRACE", "0")))
    res = run_bass_kernel_spmd(nc, in_maps, core_ids=list(range(n_cores)),
                               trace=trace)
    _last_results = res

    out = np.empty((B, 192, 32, 32), np.float32)
    for b in range(B):
        y = res.results[b]["yout"]  # [2, 96, HW]
        out[b] = y.reshape(192, 32, 32)
    return out
